# revision 22
# baseline (speedup 1.0000x reference)
"""CADIConv (GNN message passing) Trainium2 Bass kernel.

Strategy (edge/dst sharding across 8 NeuronCores):
  - Host sorts edges by destination node; nodes are split into 8 contiguous
    ranges (one per core), so every core owns all edges of its node range and
    no cross-core reduction is needed.
  - Within a core, nodes are grouped into blocks of 64; each block's edges are
    padded to a uniform subtile count so all 8 cores run one SPMD program.
  - Per 128-edge subtile the device computes
        w_rel = edge_attr @ wr_w.T + wr_b          (DVE tensor_tensor_reduce)
        t1 = x[src] * attn                         (DVE tensor_scalar)
        t2 = edge_attr * w_rel                     (DVE tensor_scalar)
        S[e, n] = (dst_local[e] == n)              (DVE tensor_scalar is_equal)
        pre = t1 + t2                              (PE identity matmuls in PSUM)
        msg = relu(pre)                            (ACT, PSUM -> SBUF)
        agg_t[h, n] += msg.T @ S                   (PE, accumulated in PSUM)
    and per block
        out_t = nn_w.T.T @ (agg_t + (1+eps)*x_t) + nn_b   (PE + ACT)
  - x[src] is fetched with SWDGE dma_gather (int16 indices), with x split at
    row 32768 into two gather sources so indices fit in int16.
  - Outputs: y_t [64, nodes] per core (host transposes/concats) and the
    per-edge w_rel (host scatters back to original edge order).
"""

import math

import numpy as np

import concourse.bacc as bacc
import concourse.mybir as mybir
import concourse.tile as tile
from concourse.bass_utils import run_bass_kernel_spmd

N_CORES = 8
NB = 64         # nodes per aggregation block
XSPLIT = 32768  # int16-addressable row limit for gather sources
P = 128         # partitions / subtile size

F32 = mybir.dt.float32
I16 = mybir.dt.int16

_PROGRAM_CACHE = {}
LAST_RESULT = None  # BassKernelResults of the most recent run (for profiling)
_LAST_RUN = None  # (nc, in_maps) of the most recent kernel() call (for benchmark)


def _ceil_div(a, b):
    return -(-a // b)


def _build_program(H, n_blocks, T_lo, T_hi, npc_pad, lo_rows, hi_rows, wr_b_val):
    import os
    STAGE = int(os.environ.get("K_STAGE", "9"))
    N_QUEUES = int(os.environ.get("K_QUEUES", "4"))
    T_blk = T_lo + T_hi
    n_sub = n_blocks * T_blk
    nc = bacc.Bacc(
        "TRN2", target_bir_lowering=False, num_swdge_queues=N_QUEUES
    )

    ea_d = nc.dram_tensor("ea_w", [P, n_sub * H], F32, kind="ExternalInput")
    attn_d = nc.dram_tensor("attn_w", [P, n_sub], F32, kind="ExternalInput")
    dst_d = nc.dram_tensor("dst_w", [P, n_sub], F32, kind="ExternalInput")
    idxlo_d = nc.dram_tensor("idx_lo", [P, n_blocks * T_lo * 8], I16, kind="ExternalInput")
    if T_hi:
        idxhi_d = nc.dram_tensor("idx_hi", [P, n_blocks * T_hi * 8], I16, kind="ExternalInput")
    xlo_d = nc.dram_tensor("x_lo", [lo_rows, H], F32, kind="ExternalInput")
    if T_hi:
        xhi_d = nc.dram_tensor("x_hi", [hi_rows, H], F32, kind="ExternalInput")
    xt_d = nc.dram_tensor("xt_pre", [H, npc_pad], F32, kind="ExternalInput")
    iota_d = nc.dram_tensor("iota64", [P, NB], F32, kind="ExternalInput")
    wrw_d = nc.dram_tensor("wrw_b", [P, H], F32, kind="ExternalInput")
    i128_d = nc.dram_tensor("i128", [P, P], F32, kind="ExternalInput")
    nnwt_d = nc.dram_tensor("nnw_t", [H, H], F32, kind="ExternalInput")
    nnb_d = nc.dram_tensor("nnb", [H, 1], F32, kind="ExternalInput")

    yt_o = nc.dram_tensor("y_t", [H, npc_pad], F32, kind="ExternalOutput")
    wrel_o = nc.dram_tensor("wrel_o", [P, n_sub], F32, kind="ExternalOutput")

    mult = mybir.AluOpType.mult
    add = mybir.AluOpType.add
    is_eq = mybir.AluOpType.is_equal
    relu = mybir.ActivationFunctionType.Relu

    with tile.TileContext(nc) as tc:
        with (
            tc.tile_pool(name="const", bufs=1) as cpool,
            tc.tile_pool(name="work", bufs=2) as wpool,
            tc.tile_pool(name="psum", bufs=2, space="PSUM") as ppool,
        ):
            iota_t = cpool.tile([P, NB], F32)
            nc.sync.dma_start(out=iota_t[:], in_=iota_d[:])
            wrw_t = cpool.tile([P, H], F32)
            nc.sync.dma_start(out=wrw_t[:], in_=wrw_d[:])
            i128_t = cpool.tile([P, P], F32)
            nc.sync.dma_start(out=i128_t[:], in_=i128_d[:])
            nnwt_t = cpool.tile([H, H], F32)
            nc.sync.dma_start(out=nnwt_t[:], in_=nnwt_d[:])
            nnb_t = cpool.tile([H, 1], F32)
            nc.sync.dma_start(out=nnb_t[:], in_=nnb_d[:])
            xt_t = cpool.tile([H, npc_pad], F32)
            nc.sync.dma_start(out=xt_t[:], in_=xt_d[:])
            attn_t = cpool.tile([P, n_sub], F32)
            nc.sync.dma_start(out=attn_t[:], in_=attn_d[:])
            dst_t = cpool.tile([P, n_sub], F32)
            nc.sync.dma_start(out=dst_t[:], in_=dst_d[:])
            idxlo_t = cpool.tile([P, n_blocks * T_lo * 8], I16)
            nc.sync.dma_start(out=idxlo_t[:], in_=idxlo_d[:])
            if T_hi:
                idxhi_t = cpool.tile([P, n_blocks * T_hi * 8], I16)
                nc.sync.dma_start(out=idxhi_t[:], in_=idxhi_d[:])

            wrel_t = cpool.tile([P, n_sub], F32)
            yt_t = cpool.tile([H, npc_pad], F32)
            trash_t = cpool.tile([P, H], F32)
            qctr = [0]
            if STAGE < 9:
                nc.vector.memset(wrel_t[:], 0.0)
                nc.vector.memset(yt_t[:], 0.0)

            for b in range(n_blocks):
                if STAGE < 1:
                    continue
                fcols = T_blk * H
                ea_t = wpool.tile([P, fcols], F32, tag="ea")
                nc.sync.dma_start(
                    out=ea_t[:], in_=ea_d[:, b * fcols : (b + 1) * fcols]
                )
                xg_t = wpool.tile([P, fcols], F32, tag="xg")
                xg3 = xg_t[:].rearrange("p (t h) -> p t h", h=H)
                if STAGE < 2:
                    continue

                # dma_gather caps at 1024 indices per call (Q7 idx scratch);
                # split into <=8-subtile calls, rotating SWDGE queues.
                def _gathers(src_t, idx_t_all, tile_off, T_half, col_base):
                    for c0 in range(0, T_half, 8):
                        cw = min(8, T_half - c0)
                        nc.gpsimd.dma_gather(
                            xg3[:, tile_off + c0 : tile_off + c0 + cw, :],
                            src_t[:],
                            idx_t_all[:, col_base + c0 * 8 : col_base + (c0 + cw) * 8],
                            cw * P,
                            cw * P,
                            H,
                            queue_num=qctr[0] % N_QUEUES,
                        )
                        qctr[0] += 1

                _gathers(xlo_d, idxlo_t, 0, T_lo, b * T_lo * 8)
                if T_hi:
                    _gathers(xhi_d, idxhi_t, T_lo, T_hi, b * T_hi * 8)

                t1_t = wpool.tile([P, fcols], F32, tag="t1")
                t2_t = wpool.tile([P, fcols], F32, tag="t2")
                s_t = wpool.tile([P, fcols], F32, tag="s")
                msg_t = wpool.tile([P, fcols], F32, tag="msg")

                if STAGE < 3:
                    continue
                # w_rel per subtile: (tensor_tensor_reduce crashes on HW; use
                # tensor_tensor + tensor_reduce instead)
                for s in range(T_blk):
                    g = b * T_blk + s
                    sl = slice(s * H, (s + 1) * H)
                    nc.vector.tensor_tensor(
                        out=trash_t[:], in0=ea_t[:, sl], in1=wrw_t[:], op=mult
                    )
                    nc.vector.tensor_reduce(
                        out=wrel_t[:, g : g + 1],
                        in_=trash_t[:],
                        axis=mybir.AxisListType.X,
                        op=add,
                    )
                if wr_b_val != 0.0:
                    bsl = slice(b * T_blk, (b + 1) * T_blk)
                    nc.scalar.add(
                        out=wrel_t[:, bsl], in_=wrel_t[:, bsl], add=float(wr_b_val)
                    )
                for s in range(T_blk):
                    g = b * T_blk + s
                    sl = slice(s * H, (s + 1) * H)
                    nc.vector.tensor_scalar(
                        out=t1_t[:, sl], in0=xg_t[:, sl],
                        scalar1=attn_t[:, g : g + 1], scalar2=None, op0=mult,
                    )
                    nc.vector.tensor_scalar(
                        out=t2_t[:, sl], in0=ea_t[:, sl],
                        scalar1=wrel_t[:, g : g + 1], scalar2=None, op0=mult,
                    )
                    nc.vector.tensor_scalar(
                        out=s_t[:, sl], in0=iota_t[:],
                        scalar1=dst_t[:, g : g + 1], scalar2=None, op0=is_eq,
                    )

                if STAGE < 4:
                    continue
                # pre = t1 + t2 on PE (identity matmuls into PSUM), relu on ACT
                for c0 in range(0, fcols, 512):
                    cw = min(512, fcols - c0)
                    pre_p = ppool.tile([P, 512], F32, tag="pre")
                    nc.tensor.matmul(
                        out=pre_p[:, :cw], lhsT=i128_t[:],
                        rhs=t1_t[:, c0 : c0 + cw], start=True, stop=False,
                    )
                    nc.tensor.matmul(
                        out=pre_p[:, :cw], lhsT=i128_t[:],
                        rhs=t2_t[:, c0 : c0 + cw], start=False, stop=True,
                    )
                    nc.scalar.activation(
                        out=msg_t[:, c0 : c0 + cw], in_=pre_p[:, :cw], func=relu
                    )

                if STAGE < 5:
                    continue
                # aggregation: agg_t[h, n] += msg_s.T @ S_s
                agg_p = ppool.tile([H, NB], F32, tag="agg")
                for s in range(T_blk):
                    sl = slice(s * H, (s + 1) * H)
                    nc.tensor.matmul(
                        out=agg_p[:],
                        lhsT=msg_t[:, sl],
                        rhs=s_t[:, sl],
                        start=(s == 0),
                        stop=(s == T_blk - 1),
                    )
                agg2_t = wpool.tile([H, NB], F32, tag="agg2")
                nc.vector.tensor_tensor(
                    out=agg2_t[:], in0=agg_p[:],
                    in1=xt_t[:, b * NB : (b + 1) * NB], op=add,
                )
                y_p = ppool.tile([H, NB], F32, tag="y")
                nc.tensor.matmul(
                    out=y_p[:], lhsT=nnwt_t[:], rhs=agg2_t[:], start=True, stop=True
                )
                nc.scalar.add(
                    out=yt_t[:, b * NB : (b + 1) * NB], in_=y_p[:], add=nnb_t[:, 0:1]
                )

            nc.sync.dma_start(out=yt_o[:], in_=yt_t[:])
            nc.sync.dma_start(out=wrel_o[:], in_=wrel_t[:])

    nc.compile()
    return nc


def kernel(x, edge_index, edge_attr, attn, nn_w, nn_b, wr_w, wr_b, eps):
    global LAST_RESULT
    x = np.asarray(x, dtype=np.float32)
    edge_attr = np.asarray(edge_attr, dtype=np.float32)
    attn = np.asarray(attn, dtype=np.float32)
    nn_w = np.asarray(nn_w, dtype=np.float32)
    nn_b = np.asarray(nn_b, dtype=np.float32)
    wr_w = np.asarray(wr_w, dtype=np.float32)
    wr_b = np.asarray(wr_b, dtype=np.float32)
    eps_f = float(np.asarray(eps))

    N, H = x.shape
    E = edge_index.shape[1]
    assert H == 64, "kernel specialized for H=64"

    src = np.asarray(edge_index[0], dtype=np.int64)
    dst = np.asarray(edge_index[1], dtype=np.int64)

    npc = _ceil_div(N, N_CORES)           # nodes per core
    n_blocks = _ceil_div(npc, NB)         # blocks per core
    npc_pad = n_blocks * NB

    core = dst // npc
    nloc = dst - core * npc
    blk = nloc // NB
    dloc = nloc - blk * NB
    hi_flag = (src >= XSPLIT).astype(np.int64)

    gkey = (core * n_blocks + blk) * 2 + hi_flag
    order = np.argsort(gkey, kind="stable")
    counts = np.bincount(gkey, minlength=N_CORES * n_blocks * 2)
    counts2 = counts.reshape(N_CORES, n_blocks, 2)
    T_lo = max(1, _ceil_div(int(counts2[:, :, 0].max()), P))
    max_hi = int(counts2[:, :, 1].max())
    T_hi = _ceil_div(max_hi, P)
    T_blk = T_lo + T_hi
    n_sub = n_blocks * T_blk

    gstart = np.zeros(counts.size + 1, dtype=np.int64)
    np.cumsum(counts, out=gstart[1:])
    gk_s = gkey[order]
    rank = np.arange(E, dtype=np.int64) - gstart[gk_s]
    c_s = core[order]
    b_s = blk[order]
    h_s = hi_flag[order]
    slot = b_s * (T_blk * P) + h_s * (T_lo * P) + rank
    t_s = slot // P
    p_s = slot % P

    # partition-major wrapped per-core arrays
    ea_w = np.zeros((N_CORES, P, n_sub, H), dtype=np.float32)
    ea_w[c_s, p_s, t_s] = edge_attr[order]
    attn_w = np.zeros((N_CORES, P, n_sub), dtype=np.float32)
    attn_w[c_s, p_s, t_s] = attn[order, 0]
    dst_w = np.zeros((N_CORES, P, n_sub), dtype=np.float32)
    dst_w[c_s, p_s, t_s] = dloc[order].astype(np.float32)

    # gather index tables (int16, wrapped in 16 partitions, replicated to 128)
    src_s = src[order]

    def _mk_tbl(T_half, mask, values):
        if T_half == 0:
            return None
        tbl = np.zeros((N_CORES, n_blocks, T_half * P), dtype=np.int16)
        tbl[c_s[mask], b_s[mask], rank[mask]] = values.astype(np.int16)
        tbl = tbl.reshape(N_CORES, n_blocks, T_half * 8, 16)
        tbl = tbl.transpose(0, 3, 1, 2).reshape(N_CORES, 16, n_blocks * T_half * 8)
        return np.tile(tbl, (1, 8, 1))

    lo_m = h_s == 0
    idx_lo = _mk_tbl(T_lo, lo_m, src_s[lo_m])
    hi_m = ~lo_m
    idx_hi = _mk_tbl(T_hi, hi_m, src_s[hi_m] - XSPLIT)

    x_lo = np.ascontiguousarray(x[:XSPLIT])
    lo_rows = x_lo.shape[0]
    hi_rows = max(1, N - XSPLIT)
    x_hi = (
        np.ascontiguousarray(x[XSPLIT:])
        if N > XSPLIT
        else np.zeros((1, H), dtype=np.float32)
    )

    # (1+eps) * x^T per core, padded to npc_pad columns
    xt_pre = np.zeros((N_CORES, H, npc_pad), dtype=np.float32)
    scale = np.float32(1.0 + eps_f)
    for c in range(N_CORES):
        n0 = c * npc
        n1 = min(N, n0 + npc)
        if n1 > n0:
            xt_pre[c, :, : n1 - n0] = (scale * x[n0:n1]).T

    iota64 = np.tile(np.arange(NB, dtype=np.float32), (P, 1))
    wrw_b = np.tile(wr_w.reshape(1, H).astype(np.float32), (P, 1))
    i128 = np.eye(P, dtype=np.float32)
    nnw_t = np.ascontiguousarray(nn_w.T)
    nnb = np.ascontiguousarray(nn_b.reshape(H, 1))

    key = (H, n_blocks, T_lo, T_hi, npc_pad, lo_rows, hi_rows, float(wr_b[0]))
    nc = _PROGRAM_CACHE.get(key)
    if nc is None:
        nc = _build_program(
            H, n_blocks, T_lo, T_hi, npc_pad, lo_rows, hi_rows, float(wr_b[0])
        )
        _PROGRAM_CACHE[key] = nc

    in_maps = []
    for c in range(N_CORES):
        m = {
            "ea_w": ea_w[c].reshape(P, n_sub * H),
            "attn_w": attn_w[c],
            "dst_w": dst_w[c],
            "idx_lo": idx_lo[c],
            "x_lo": x_lo,
            "xt_pre": xt_pre[c],
            "iota64": iota64,
            "wrw_b": wrw_b,
            "i128": i128,
            "nnw_t": nnw_t,
            "nnb": nnb,
        }
        if T_hi:
            m["idx_hi"] = idx_hi[c]
            m["x_hi"] = x_hi
        in_maps.append(m)

    import os as _os
    global _LAST_RUN
    _LAST_RUN = (nc, in_maps)
    _ncores_run = int(_os.environ.get("K_CORES", str(N_CORES)))
    res = run_bass_kernel_spmd(
        nc, in_maps[:_ncores_run], core_ids=list(range(_ncores_run))
    )
    if _ncores_run < N_CORES:

        class _R:
            results = list(res.results) + [res.results[0]] * (N_CORES - _ncores_run)
            exec_time_ns = res.exec_time_ns

        res = _R()
    LAST_RESULT = res

    # assemble outputs
    y = np.empty((N, H), dtype=np.float32)
    for c in range(N_CORES):
        n0 = c * npc
        n1 = min(N, n0 + npc)
        if n1 > n0:
            y[n0:n1] = res.results[c]["y_t"][:, : n1 - n0].T

    wrel_stack = np.stack([res.results[c]["wrel_o"] for c in range(N_CORES)])
    w_rel = np.empty((E, 1), dtype=np.float32)
    w_rel[order, 0] = wrel_stack[c_s, p_s, t_s]

    return y, w_rel


def benchmark(iters=8):
    """Re-execute the last-built NEFF with device-resident inputs and return
    per-execution wall times (seconds). Mirrors bass2jax.run_bass_via_pjrt's
    multi-core path but keeps inputs on device and reuses the jitted callable,
    so steady-state iterations measure NEFF execution + dispatch only."""
    import time

    import jax
    from jax.experimental.shard_map import shard_map
    from jax.sharding import Mesh, NamedSharding, PartitionSpec

    import concourse.bass2jax as b2j
    import concourse.mybir as mb

    assert _LAST_RUN is not None, "call kernel() first"
    nc, in_maps = _LAST_RUN
    n_cores = len(in_maps)
    b2j.install_neuronx_cc_hook()

    partition_name = nc.partition_id_tensor.name if nc.partition_id_tensor else None
    in_names, out_names, out_avals, zero_outs = [], [], [], []
    for alloc in nc.m.functions[0].allocations:
        if not isinstance(alloc, mb.MemoryLocationSet):
            continue
        name = alloc.memorylocations[0].name
        if alloc.kind == "ExternalInput":
            if name != partition_name:
                in_names.append(name)
        elif alloc.kind == "ExternalOutput":
            out_names.append(name)
            shape = tuple(alloc.tensor_shape)
            dtype = mb.dt.np(alloc.dtype)
            out_avals.append(jax.core.ShapedArray(shape, dtype))
            zero_outs.append(np.zeros(shape, dtype))
    n_params = len(in_names)
    all_names = in_names + out_names
    if partition_name is not None:
        all_names.append(partition_name)

    def _body(*args):
        operands = list(args)
        if partition_name is not None:
            operands.append(b2j.partition_id_tensor())
        return tuple(
            b2j._bass_exec_p.bind(
                *operands,
                out_avals=tuple(out_avals),
                in_names=tuple(all_names),
                out_names=tuple(out_names),
                lowering_input_output_aliases=(),
                sim_require_finite=True,
                sim_require_nnan=True,
                nc=nc,
            )
        )

    devices = jax.devices()[:n_cores]
    mesh = Mesh(np.asarray(devices), ("core",))
    nsh = NamedSharding(mesh, PartitionSpec("core"))
    in_specs = (PartitionSpec("core"),) * (n_params + len(zero_outs))
    out_specs = (PartitionSpec("core"),) * len(out_names)
    sharded = jax.jit(
        shard_map(_body, mesh=mesh, in_specs=in_specs, out_specs=out_specs,
                  check_rep=False),
        keep_unused=True,
    )
    concat_in = [
        jax.device_put(
            np.concatenate([np.asarray(m[n]) for m in in_maps], axis=0), nsh
        )
        for n in in_names
    ]
    concat_zeros = [
        jax.device_put(np.zeros((n_cores * z.shape[0], *z.shape[1:]), z.dtype), nsh)
        for z in zero_outs
    ]
    times = []
    for _ in range(iters):
        t0 = time.perf_counter()
        outs = sharded(*concat_in, *concat_zeros)
        jax.block_until_ready(outs)
        times.append(time.perf_counter() - t0)
    return times
